# revision 7
# baseline (speedup 1.0000x reference)
"""Trainium2 Bass kernel for BlockUncertaintyTracker (segment_reduce).

Computes, per 4x4 block of a [16,1,2048,2048] image batch:
  - mean over the 16 block elements, averaged over batch
  - 0.9-quantile (= 0.5*(2nd largest + 3rd largest)), averaged over batch
  - EMA update of both stats, then broadcasts the ratio back to full shape.

Sharding: spatial over H across 8 cores (256 image rows / 64 block rows per
core). Every core sees all 16 batch elements for its rows, so no collectives
are needed; EMA buffer slices are contiguous per core.
"""

import os

import numpy as np

# ---- problem constants (hardcoded; kernel.py must be self-contained) ----
B = 16          # batch
H = 2048
W = 2048
BS = 4          # block size
NCORES = 8
HS = H // NCORES            # 256 rows per core
NBH = HS // BS              # 64 block rows per core
NBW = W // BS               # 512 block cols
ROWS = B * HS               # 4096 rows in a per-core slab
NGROUPS = 8                 # groups per core; each = 2 batches x 256 rows
GB = B // NGROUPS           # 2 batches per group
DECAY = 0.99
ALPHA = 0.1
EPS = 1e-5
C_MEAN = (1.0 - DECAY) / (BS * BS * B)    # fold mean-over-16-elems and batch
C_QUANT = (1.0 - DECAY) * 0.5 / B         # fold 0.5*(m2+m3) and batch mean

_CACHE = {}


def _split_multi_waits(nc):
    """This walrus build encodes at most ONE sync wait per instruction.
    Tile attaches several. Hoist excess waits onto same-engine NOPs placed
    immediately before the owning instruction (same engine stream => same
    semantics)."""
    import concourse.mybir as mybir

    plans = []  # (bb, inst, extra_waits)
    for f in nc.m.functions:
        for bb in f.blocks:
            for inst in bb.instructions:
                si = getattr(inst, "sync_info", None)
                waits = list(si.on_wait) if (si and si.on_wait) else []
                if len(waits) > 1:
                    si.on_wait = [waits[-1]]
                    plans.append((bb, inst.name, waits[:-1]))

    if not plans:
        return

    # Create the nops (they get appended to nc's current block; we re-home
    # them below), one per extra wait.
    nop_for = {}
    stray = set()
    for bb, iname, extra in plans:
        nops = []
        for w in extra:
            nop = nc.engines[nc.inst_map[iname].engine].nop(nofuse=True).ins
            nop.sync_info = mybir.SyncInfo(on_wait=[w], on_update=[])
            nops.append(nop)
            stray.add(nop.name)
        nop_for[iname] = nops

    for f in nc.m.functions:
        for bb in f.blocks:
            out = []
            changed = False
            for inst in bb.instructions:
                if inst.name in stray and inst.name not in {
                    n.name for ns in nop_for.values() for n in ns
                }:
                    changed = True
                    continue
                if inst.name in stray:
                    # a stray auto-appended copy: drop it here
                    changed = True
                    continue
                if inst.name in nop_for:
                    out.extend(nop_for[inst.name])
                    changed = True
                out.append(inst)
            if changed:
                bb.instructions = out


def _build():
    """Builds the single-core Bass program (SPMD across 8 cores)."""
    from contextlib import ExitStack

    import concourse.bass as bass
    import concourse.mybir as mybir
    import concourse.tile as tile

    f32 = mybir.dt.float32
    MAX = mybir.AluOpType.max
    MIN = mybir.AluOpType.min
    MULT = mybir.AluOpType.mult
    ADD = mybir.AluOpType.add
    X = mybir.AxisListType.X

    nc = bass.Bass("TRN2", target_bir_lowering=False, debug=False)

    x = nc.dram_tensor("x", [ROWS, W], f32, kind="ExternalInput").ap()
    ee = nc.dram_tensor("ee", [NBH, NBW], f32, kind="ExternalInput").ap()
    eq = nc.dram_tensor("eq", [NBH, NBW], f32, kind="ExternalInput").ap()
    ones2 = nc.dram_tensor("ones2", [128, NBH], f32, kind="ExternalInput").ap()
    y = nc.dram_tensor("y", [ROWS, W], f32, kind="ExternalOutput").ap()

    # [16, 4, 64, 2048]: batch, row-in-block (r), block-row (i), col
    x4 = x.rearrange("(b i r) w -> b r i w", b=B, i=NBH, r=BS)
    y4 = y.rearrange("(b i r) w -> b i r w", b=B, i=NBH, r=BS)

    with tile.TileContext(nc) as tc, ExitStack() as ctx:
        pool = ctx.enter_context(tc.tile_pool(name="work", bufs=1))
        ppool = ctx.enter_context(tc.tile_pool(name="acc", bufs=1, space="PSUM"))

        psum_s = ppool.tile([NBH, NBW], f32, tag="ps")
        psum_q = ppool.tile([NBH, NBW], f32, tag="pq")

        ones_sb = pool.tile([128, NBH], f32, tag="ones")
        nc.sync.dma_start(ones_sb[:, :], ones2)

        def tt(dst, a, bb, op):
            nc.vector.tensor_tensor(dst, a, bb, op)

        for g in range(NGROUPS):
            rts = []
            for r in range(BS):
                rt = pool.tile([128, W], f32, tag=f"r{r}", bufs=2)
                # two batch halves -> partitions 0:64 and 64:128
                nc.sync.dma_start(rt[0:64, :], x4[GB * g, r, :, :])
                nc.sync.dma_start(rt[64:128, :], x4[GB * g + 1, r, :, :])
                rts.append(rt)
            r0, r1, r2, r3 = rts

            # sum path (f32 exact): Vs = r0+r1+r2+r3, then window-4 sum
            s01 = pool.tile([128, W], f32, tag="big", bufs=8)
            tt(s01[:, :], r0[:, :], r1[:, :], ADD)
            s23 = pool.tile([128, W], f32, tag="big", bufs=8)
            tt(s23[:, :], r2[:, :], r3[:, :], ADD)

            # vertical compare-exchange pairs
            v1 = pool.tile([128, W], f32, tag="big", bufs=8)
            tt(v1[:, :], r0[:, :], r1[:, :], MAX)
            w1v = pool.tile([128, W], f32, tag="big", bufs=8)
            tt(w1v[:, :], r0[:, :], r1[:, :], MIN)
            v2 = pool.tile([128, W], f32, tag="big", bufs=8)
            tt(v2[:, :], r2[:, :], r3[:, :], MAX)
            w2v = pool.tile([128, W], f32, tag="big", bufs=8)
            tt(w2v[:, :], r2[:, :], r3[:, :], MIN)

            vs = pool.tile([128, W], f32, tag="big", bufs=8)
            tt(vs[:, :], s01[:, :], s23[:, :], ADD)
            ws = pool.tile([128, NBW], f32, tag="ws", bufs=2)
            nc.vector.reduce_sum(
                ws[:, :], vs.rearrange("p (j c) -> p j c", c=BS), axis=X
            )

            # vertical sorted-3: M >= S2 >= T3 per column
            m = pool.tile([128, W], f32, tag="big", bufs=8)
            tt(m[:, :], v1[:, :], v2[:, :], MAX)
            t1 = pool.tile([128, W], f32, tag="big", bufs=8)
            tt(t1[:, :], v1[:, :], v2[:, :], MIN)
            t2 = pool.tile([128, W], f32, tag="big", bufs=8)
            tt(t2[:, :], w1v[:, :], w2v[:, :], MAX)
            s2 = pool.tile([128, W], f32, tag="big", bufs=8)
            tt(s2[:, :], t1[:, :], t2[:, :], MAX)
            t3 = pool.tile([128, W], f32, tag="big", bufs=8)
            tt(t3[:, :], t1[:, :], t2[:, :], MIN)

            def eo(ap):
                v = ap.rearrange("p (j two) -> p j two", two=2)
                return v[:, :, 0], v[:, :, 1]

            me, mo = eo(m)
            s2e, s2o = eo(s2)
            t3e, t3o = eo(t3)

            # level A: merge adjacent-column sorted-3 pairs -> top-3 (p1,p2,p3)
            HW2 = W // 2
            p1 = pool.tile([128, HW2], f32, tag="mid", bufs=7)
            tt(p1[:, :], me, mo, MAX)
            u1 = pool.tile([128, HW2], f32, tag="mid", bufs=7)
            tt(u1[:, :], me, mo, MIN)
            u2 = pool.tile([128, HW2], f32, tag="mid", bufs=7)
            tt(u2[:, :], s2e, s2o, MAX)
            p2 = pool.tile([128, HW2], f32, tag="mid", bufs=7)
            tt(p2[:, :], u1[:, :], u2[:, :], MAX)
            w1 = pool.tile([128, HW2], f32, tag="mid", bufs=7)
            tt(w1[:, :], t3e, t3o, MAX)
            w2 = pool.tile([128, HW2], f32, tag="mid", bufs=7)
            tt(w2[:, :], me, s2o, MIN)
            w3 = pool.tile([128, HW2], f32, tag="mid", bufs=7)
            tt(w3[:, :], s2e, mo, MIN)
            w4 = pool.tile([128, HW2], f32, tag="mid", bufs=7)
            tt(w4[:, :], w2[:, :], w3[:, :], MAX)
            p3 = pool.tile([128, HW2], f32, tag="mid", bufs=7)
            tt(p3[:, :], w1[:, :], w4[:, :], MAX)

            # level B: merge the two pair-lists -> 2nd + 3rd largest of 16
            p1e, p1o = eo(p1)
            p2e, p2o = eo(p2)
            p3e, p3o = eo(p3)
            z1 = pool.tile([128, NBW], f32, tag="small", bufs=6)
            tt(z1[:, :], p1e, p1o, MIN)
            z2 = pool.tile([128, NBW], f32, tag="small", bufs=6)
            tt(z2[:, :], p2e, p2o, MAX)
            c2 = pool.tile([128, NBW], f32, tag="small", bufs=6)
            tt(c2[:, :], z1[:, :], z2[:, :], MAX)
            z3 = pool.tile([128, NBW], f32, tag="small", bufs=6)
            tt(z3[:, :], p3e, p3o, MAX)
            z4 = pool.tile([128, NBW], f32, tag="small", bufs=6)
            tt(z4[:, :], p1e, p2o, MIN)
            z5 = pool.tile([128, NBW], f32, tag="small", bufs=6)
            tt(z5[:, :], p2e, p1o, MIN)
            z6 = pool.tile([128, NBW], f32, tag="small", bufs=6)
            tt(z6[:, :], z4[:, :], z5[:, :], MAX)
            c3 = pool.tile([128, NBW], f32, tag="small", bufs=6)
            tt(c3[:, :], z3[:, :], z6[:, :], MAX)
            qs = pool.tile([128, NBW], f32, tag="qs", bufs=2)
            tt(qs[:, :], c2[:, :], c3[:, :], ADD)

            # batch accumulation: psum[m, n] += sum_p ones[p, m] * stat[p, n]
            nc.tensor.matmul(
                psum_s[:, :], lhsT=ones_sb[:, :], rhs=ws[:, :],
                start=(g == 0), stop=(g == NGROUPS - 1),
            )
            nc.tensor.matmul(
                psum_q[:, :], lhsT=ones_sb[:, :], rhs=qs[:, :],
                start=(g == 0), stop=(g == NGROUPS - 1),
            )

        # ---- tail: EMA update + ratio + broadcast ----
        ee_sb = pool.tile([NBH, NBW], f32, tag="ee")
        nc.sync.dma_start(ee_sb[:, :], ee)
        eq_sb = pool.tile([NBH, NBW], f32, tag="eq")
        nc.sync.dma_start(eq_sb[:, :], eq)

        ee2 = pool.tile([NBH, NBW], f32, tag="ee2")
        nc.vector.tensor_scalar(ee2[:, :], ee_sb[:, :], DECAY, EPS, MULT, ADD)
        eq2 = pool.tile([NBH, NBW], f32, tag="eq2")
        nc.vector.tensor_scalar(eq2[:, :], eq_sb[:, :], DECAY, 0.0, MULT, ADD)

        den = pool.tile([NBH, NBW], f32, tag="den")
        nc.vector.scalar_tensor_tensor(
            den[:, :], psum_s[:, :], C_MEAN, ee2[:, :], op0=MULT, op1=ADD
        )
        num = pool.tile([NBH, NBW], f32, tag="num")
        nc.vector.scalar_tensor_tensor(
            num[:, :], psum_q[:, :], C_QUANT, eq2[:, :], op0=MULT, op1=ADD
        )
        rec = pool.tile([NBH, NBW], f32, tag="rec")
        nc.vector.reciprocal(rec[:, :], den[:, :])
        u = pool.tile([NBH, NBW], f32, tag="u")
        nc.vector.tensor_tensor(u[:, :], num[:, :], rec[:, :], MULT)

        # expand x4 along columns: u4[p, j*4 + c] = u[p, j]
        u4 = pool.tile([NBH, W], f32, tag="u4")
        u4v = u4.rearrange("p (j c) -> p j c", c=BS)
        for c in range(BS):
            nc.vector.tensor_copy(u4v[:, :, c], u[:, :])

        # replicate to 4 rows x 16 batches
        u4b = u4[:, :].unsqueeze(1).broadcast_to((NBH, BS, W))
        for b in range(B):
            nc.sync.dma_start(y4[b], u4b)

    _split_multi_waits(nc)
    return nc


def _get_nc():
    if "nc" not in _CACHE:
        _CACHE["nc"] = _build()
    return _CACHE["nc"]


def kernel(current_errors, ema_errors, ema_quantile):
    from concourse.bass_utils import run_bass_kernel_spmd

    x = np.asarray(current_errors, dtype=np.float32).reshape(B, H, W)
    ee = np.asarray(ema_errors, dtype=np.float32).reshape(H // BS, W // BS)
    eq = np.asarray(ema_quantile, dtype=np.float32).reshape(H // BS, W // BS)

    ones2 = np.zeros((128, NBH), dtype=np.float32)
    ones2[np.arange(128), np.arange(128) % NBH] = 1.0

    in_maps = []
    for k in range(NCORES):
        xs = np.ascontiguousarray(x[:, k * HS : (k + 1) * HS, :]).reshape(ROWS, W)
        ees = np.ascontiguousarray(ee[k * NBH : (k + 1) * NBH, :])
        eqs = np.ascontiguousarray(eq[k * NBH : (k + 1) * NBH, :])
        in_maps.append({"x": xs, "ee": ees, "eq": eqs, "ones2": ones2})

    nc = _get_nc()
    trace = bool(int(os.environ.get("KERNEL_TRACE", "0")))
    res = run_bass_kernel_spmd(
        nc, in_maps, core_ids=list(range(NCORES)), trace=trace
    )
    _CACHE["last_results"] = res

    out = np.empty((B, 1, H, W), dtype=np.float32)
    for k in range(NCORES):
        out[:, 0, k * HS : (k + 1) * HS, :] = res.results[k]["y"].reshape(B, HS, W)
    return out


# revision 10
# speedup vs baseline: 1.4996x; 1.4996x over previous
"""Trainium2 Bass kernel for BlockUncertaintyTracker (segment_reduce).

Computes, per 4x4 block of a [16,1,2048,2048] image batch:
  - mean over the 16 block elements, averaged over batch
  - 0.9-quantile (= 0.5*(2nd largest + 3rd largest)), averaged over batch
  - EMA update of both stats, then broadcasts the ratio back to full shape.

Sharding: spatial over H across 8 cores (256 image rows / 64 block rows per
core). Every core sees all 16 batch elements for its rows, so no collectives
are needed; EMA buffer slices are contiguous per core.

Engine split per group of 2 batches (4 row-phase tiles R_r [128,2048] f32):
  - ScalarE: cast R_r -> fp16, and even/odd de-interleaves between merge
    levels so every DVE tensor_tensor runs contiguous step-1 fp16 (2x mode).
  - VectorE: vertical sorted-3 across the 4 row tiles, then two merge levels
    down to (2nd+3rd largest) per block.
  - TensorE: block sums via 16 strided-rhs f32 matmuls against a 0/1 matrix
    (exact f32 mean path) + batch accumulation of the quantile stat in PSUM.
  - Output: ratio computed in a row-duplicated [128,512] layout, expanded and
    written as 32 full-partition 1 MiB DMAs.
"""

import os

import numpy as np

# ---- problem constants (hardcoded; kernel.py must be self-contained) ----
B = 16          # batch
H = 2048
W = 2048
BS = 4          # block size
NCORES = 8
HS = H // NCORES            # 256 rows per core
NBH = HS // BS              # 64 block rows per core
NBW = W // BS               # 512 block cols
ROWS = B * HS               # 4096 rows in a per-core slab
NGROUPS = 8                 # groups per core; each = 2 batches x 256 rows
GB = B // NGROUPS           # 2 batches per group
DECAY = 0.99
ALPHA = 0.1
EPS = 1e-5
C_MEAN = (1.0 - DECAY) / (BS * BS * B)    # fold mean-over-16-elems and batch
C_QUANT = (1.0 - DECAY) * 0.5 / B         # fold 0.5*(m2+m3) and batch mean

_CACHE = {}


def _split_multi_waits(nc):
    """This walrus build encodes at most ONE sync wait per instruction.
    Tile attaches several. Hoist excess waits onto same-engine NOPs placed
    immediately before the owning instruction (same engine stream => same
    semantics)."""
    import concourse.mybir as mybir

    plans = []  # (inst_name, extra_waits)
    for f in nc.m.functions:
        for bb in f.blocks:
            for inst in bb.instructions:
                si = getattr(inst, "sync_info", None)
                waits = list(si.on_wait) if (si and si.on_wait) else []
                if len(waits) > 1:
                    si.on_wait = [waits[-1]]
                    plans.append((inst.name, waits[:-1]))

    if not plans:
        return

    nop_for = {}
    stray = set()
    for iname, extra in plans:
        nops = []
        for w in extra:
            nop = nc.engines[nc.inst_map[iname].engine].nop(nofuse=True).ins
            nop.sync_info = mybir.SyncInfo(on_wait=[w], on_update=[])
            nops.append(nop)
            stray.add(nop.name)
        nop_for[iname] = nops

    for f in nc.m.functions:
        for bb in f.blocks:
            out = []
            changed = False
            for inst in bb.instructions:
                if inst.name in stray:
                    changed = True
                    continue
                if inst.name in nop_for:
                    out.extend(nop_for[inst.name])
                    changed = True
                out.append(inst)
            if changed:
                bb.instructions = out


def _build():
    """Builds the single-core Bass program (SPMD across 8 cores)."""
    from contextlib import ExitStack

    import concourse.bass as bass
    import concourse.mybir as mybir
    import concourse.tile as tile

    f32 = mybir.dt.float32
    f16 = mybir.dt.float16
    MAX = mybir.AluOpType.max
    MIN = mybir.AluOpType.min
    MULT = mybir.AluOpType.mult
    ADD = mybir.AluOpType.add

    nc = bass.Bass("TRN2", target_bir_lowering=False, debug=False)

    x = nc.dram_tensor("x", [ROWS, W], f32, kind="ExternalInput").ap()
    ee = nc.dram_tensor("ee", [NBH, NBW], f32, kind="ExternalInput").ap()
    eq = nc.dram_tensor("eq", [NBH, NBW], f32, kind="ExternalInput").ap()
    # ones2[p, m] = (p % 64 == m // 2): batch-pair fold + row duplication
    ones2 = nc.dram_tensor("ones2", [128, 128], f32, kind="ExternalInput").ap()
    y = nc.dram_tensor("y", [ROWS, W], f32, kind="ExternalOutput").ap()

    # input: row = ((g*2 + b2)*64 + i)*4 + r; per (g, r): [128=(b2,i), 2048]
    xr = x.rearrange("(g b2 i r) w -> g r (b2 i) w", g=NGROUPS, b2=GB, i=NBH, r=BS)
    # output: row = b*256 + 4i + 2h + r2; per (b, h): [64, 2, 2048] = 128
    # outer steps zipped against the SBUF tile's 128 partitions (p = 2i + r2)
    y5 = y.rearrange("(b i h r2) w -> b h i r2 w", b=B, i=NBH, h=2, r2=2)

    with tile.TileContext(nc) as tc, ExitStack() as ctx:
        pool = ctx.enter_context(tc.tile_pool(name="work", bufs=1))
        ppool = ctx.enter_context(tc.tile_pool(name="acc", bufs=1, space="PSUM"))

        psum_s = ppool.tile([128, NBW], f32, tag="ps")
        psum_q = ppool.tile([128, NBW], f32, tag="pq")

        ones_sb = pool.tile([128, 128], f32, tag="ones")
        nc.sync.dma_start(ones_sb[:, :], ones2)

        def tt(dst, a, bb, op):
            nc.vector.tensor_tensor(dst, a, bb, op)

        NMM = NGROUPS * BS * BS  # sum matmuls total (accumulation group size)

        for g in range(NGROUPS):
            rts = []
            for r in range(BS):
                rt = pool.tile([128, W], f32, tag=f"r{r}", bufs=2)
                nc.sync.dma_start(rt[:, :], xr[g, r])
                rts.append(rt)

            # exact f32 block sums on TensorE: psum_s[2i+d, j] +=
            #   sum_{b2} R_r[b2*64+i, 4j+c] over all (r, c)
            for r in range(BS):
                rv = rts[r].rearrange("p (j c) -> p j c", c=BS)
                for c in range(BS):
                    k = (g * BS + r) * BS + c
                    nc.tensor.matmul(
                        psum_s[:, :], lhsT=ones_sb[:, :], rhs=rv[:, :, c],
                        start=(k == 0), stop=(k == NMM - 1),
                    )

            # fp16 casts on ScalarE
            bts = []
            for r in range(BS):
                bt = pool.tile([128, W], f16, tag=f"b{r}", bufs=2)
                nc.scalar.copy(bt[:, :], rts[r][:, :])
                bts.append(bt)
            b0, b1, b2_, b3 = bts

            # vertical sorted-3 per column: m >= s2 >= t3 (fp16, 2x)
            v1 = pool.tile([128, W], f16, tag="big", bufs=8)
            tt(v1[:, :], b0[:, :], b1[:, :], MAX)
            w1v = pool.tile([128, W], f16, tag="big", bufs=8)
            tt(w1v[:, :], b0[:, :], b1[:, :], MIN)
            v2 = pool.tile([128, W], f16, tag="big", bufs=8)
            tt(v2[:, :], b2_[:, :], b3[:, :], MAX)
            w2v = pool.tile([128, W], f16, tag="big", bufs=8)
            tt(w2v[:, :], b2_[:, :], b3[:, :], MIN)
            m = pool.tile([128, W], f16, tag="big", bufs=8)
            tt(m[:, :], v1[:, :], v2[:, :], MAX)
            t1 = pool.tile([128, W], f16, tag="big", bufs=8)
            tt(t1[:, :], v1[:, :], v2[:, :], MIN)
            t2 = pool.tile([128, W], f16, tag="big", bufs=8)
            tt(t2[:, :], w1v[:, :], w2v[:, :], MAX)
            s2 = pool.tile([128, W], f16, tag="big", bufs=8)
            tt(s2[:, :], t1[:, :], t2[:, :], MAX)
            t3 = pool.tile([128, W], f16, tag="big", bufs=8)
            tt(t3[:, :], t1[:, :], t2[:, :], MIN)

            # ScalarE de-interleave of m/s2/t3 into even/odd planes
            HW2 = W // 2

            def deint(src, w_out, tag, bufs):
                v = src.rearrange("p (j two) -> p j two", two=2)
                te = pool.tile([128, w_out], f16, tag=tag, bufs=bufs, name=f"{tag}e")
                nc.scalar.copy(te[:, :], v[:, :, 0])
                to = pool.tile([128, w_out], f16, tag=tag, bufs=bufs, name=f"{tag}o")
                nc.scalar.copy(to[:, :], v[:, :, 1])
                return te, to

            me, mo = deint(m, HW2, "eoa", 7)
            s2e, s2o = deint(s2, HW2, "eoa", 7)
            t3e, t3o = deint(t3, HW2, "eoa", 7)

            # level A: merge adjacent-column sorted-3 pairs -> top-3 (fp16 2x)
            p1 = pool.tile([128, HW2], f16, tag="mid", bufs=7)
            tt(p1[:, :], me[:, :], mo[:, :], MAX)
            u1 = pool.tile([128, HW2], f16, tag="mid", bufs=7)
            tt(u1[:, :], me[:, :], mo[:, :], MIN)
            u2 = pool.tile([128, HW2], f16, tag="mid", bufs=7)
            tt(u2[:, :], s2e[:, :], s2o[:, :], MAX)
            p2 = pool.tile([128, HW2], f16, tag="mid", bufs=7)
            tt(p2[:, :], u1[:, :], u2[:, :], MAX)
            w1 = pool.tile([128, HW2], f16, tag="mid", bufs=7)
            tt(w1[:, :], t3e[:, :], t3o[:, :], MAX)
            w2 = pool.tile([128, HW2], f16, tag="mid", bufs=7)
            tt(w2[:, :], me[:, :], s2o[:, :], MIN)
            w3 = pool.tile([128, HW2], f16, tag="mid", bufs=7)
            tt(w3[:, :], s2e[:, :], mo[:, :], MIN)
            w4 = pool.tile([128, HW2], f16, tag="mid", bufs=7)
            tt(w4[:, :], w2[:, :], w3[:, :], MAX)
            p3 = pool.tile([128, HW2], f16, tag="mid", bufs=7)
            tt(p3[:, :], w1[:, :], w4[:, :], MAX)

            # ScalarE de-interleave of p1/p2/p3
            p1e, p1o = deint(p1, NBW, "eob", 7)
            p2e, p2o = deint(p2, NBW, "eob", 7)
            p3e, p3o = deint(p3, NBW, "eob", 7)

            # level B: 2nd + 3rd largest of all 16 (fp16 2x, final add f32)
            z1 = pool.tile([128, NBW], f16, tag="small", bufs=6)
            tt(z1[:, :], p1e[:, :], p1o[:, :], MIN)
            z2 = pool.tile([128, NBW], f16, tag="small", bufs=6)
            tt(z2[:, :], p2e[:, :], p2o[:, :], MAX)
            c2 = pool.tile([128, NBW], f16, tag="small", bufs=6)
            tt(c2[:, :], z1[:, :], z2[:, :], MAX)
            z3 = pool.tile([128, NBW], f16, tag="small", bufs=6)
            tt(z3[:, :], p3e[:, :], p3o[:, :], MAX)
            z4 = pool.tile([128, NBW], f16, tag="small", bufs=6)
            tt(z4[:, :], p1e[:, :], p2o[:, :], MIN)
            z5 = pool.tile([128, NBW], f16, tag="small", bufs=6)
            tt(z5[:, :], p2e[:, :], p1o[:, :], MIN)
            z6 = pool.tile([128, NBW], f16, tag="small", bufs=6)
            tt(z6[:, :], z4[:, :], z5[:, :], MAX)
            c3 = pool.tile([128, NBW], f16, tag="small", bufs=6)
            tt(c3[:, :], z3[:, :], z6[:, :], MAX)
            qs = pool.tile([128, NBW], f32, tag="qs", bufs=2)
            tt(qs[:, :], c2[:, :], c3[:, :], ADD)

            nc.tensor.matmul(
                psum_q[:, :], lhsT=ones_sb[:, :], rhs=qs[:, :],
                start=(g == 0), stop=(g == NGROUPS - 1),
            )

        # ---- tail: EMA update + ratio + broadcast (row-duplicated layout) ----
        ee_sb = pool.tile([128, NBW], f32, tag="ee")
        nc.sync.dma_start(ee_sb[:, :], ee.unsqueeze(1).broadcast_to((NBH, 2, NBW)))
        eq_sb = pool.tile([128, NBW], f32, tag="eq")
        nc.sync.dma_start(eq_sb[:, :], eq.unsqueeze(1).broadcast_to((NBH, 2, NBW)))

        ee2 = pool.tile([128, NBW], f32, tag="ee2")
        nc.vector.tensor_scalar(ee2[:, :], ee_sb[:, :], DECAY, EPS, MULT, ADD)
        eq2 = pool.tile([128, NBW], f32, tag="eq2")
        nc.vector.tensor_scalar(eq2[:, :], eq_sb[:, :], DECAY, 0.0, MULT, ADD)

        den = pool.tile([128, NBW], f32, tag="den")
        nc.vector.scalar_tensor_tensor(
            den[:, :], psum_s[:, :], C_MEAN, ee2[:, :], op0=MULT, op1=ADD
        )
        num = pool.tile([128, NBW], f32, tag="num")
        nc.vector.scalar_tensor_tensor(
            num[:, :], psum_q[:, :], C_QUANT, eq2[:, :], op0=MULT, op1=ADD
        )
        rec = pool.tile([128, NBW], f32, tag="rec")
        nc.vector.reciprocal(rec[:, :], den[:, :])
        u = pool.tile([128, NBW], f32, tag="u")
        nc.vector.tensor_tensor(u[:, :], num[:, :], rec[:, :], MULT)

        # expand x4 along columns: u4[p, j*4 + c] = u[p, j]
        u4 = pool.tile([128, W], f32, tag="u4")
        u4v = u4.rearrange("p (j c) -> p j c", c=BS)
        for c in range(BS):
            nc.vector.tensor_copy(u4v[:, :, c], u[:, :])

        # 32 full-partition writes: batch x row-pair-half
        for b in range(B):
            for h in range(2):
                nc.sync.dma_start(y5[b, h], u4[:, :])

    _split_multi_waits(nc)
    return nc


def _get_nc():
    if "nc" not in _CACHE:
        _CACHE["nc"] = _build()
    return _CACHE["nc"]


def kernel(current_errors, ema_errors, ema_quantile):
    from concourse.bass_utils import run_bass_kernel_spmd

    x = np.asarray(current_errors, dtype=np.float32).reshape(B, H, W)
    ee = np.asarray(ema_errors, dtype=np.float32).reshape(H // BS, W // BS)
    eq = np.asarray(ema_quantile, dtype=np.float32).reshape(H // BS, W // BS)

    # ones2[p, m] == 1 iff p % 64 == m // 2
    ones2 = np.zeros((128, 128), dtype=np.float32)
    p = np.arange(128)
    ones2[p, (p % NBH) * 2] = 1.0
    ones2[p, (p % NBH) * 2 + 1] = 1.0

    in_maps = []
    for k in range(NCORES):
        xs = np.ascontiguousarray(x[:, k * HS : (k + 1) * HS, :]).reshape(ROWS, W)
        ees = np.ascontiguousarray(ee[k * NBH : (k + 1) * NBH, :])
        eqs = np.ascontiguousarray(eq[k * NBH : (k + 1) * NBH, :])
        in_maps.append({"x": xs, "ee": ees, "eq": eqs, "ones2": ones2})

    nc = _get_nc()
    trace = bool(int(os.environ.get("KERNEL_TRACE", "0")))
    res = run_bass_kernel_spmd(
        nc, in_maps, core_ids=list(range(NCORES)), trace=trace
    )
    _CACHE["last_results"] = res

    out = np.empty((B, 1, H, W), dtype=np.float32)
    for k in range(NCORES):
        out[:, 0, k * HS : (k + 1) * HS, :] = res.results[k]["y"].reshape(B, HS, W)
    return out


# revision 13
# speedup vs baseline: 1.7583x; 1.1725x over previous
"""Trainium2 Bass kernel for BlockUncertaintyTracker (segment_reduce).

Computes, per 4x4 block of a [16,1,2048,2048] image batch:
  - mean over the 16 block elements, averaged over batch
  - 0.9-quantile (= 0.5*(2nd largest + 3rd largest)), averaged over batch
  - EMA update of both stats, then broadcasts the ratio back to full shape.

Sharding: spatial over H across 8 cores (256 image rows / 64 block rows per
core). Every core sees all 16 batch elements for its rows, so no collectives
are needed; EMA buffer slices are contiguous per core.

Engine split per group of 2 batches (4 row-phase tiles R_r [128,2048] f32):
  - ScalarE: cast R_r -> fp16, and even/odd de-interleaves between merge
    levels so every DVE tensor_tensor runs contiguous step-1 fp16 (2x mode).
  - VectorE: vertical sorted-3 across the 4 row tiles, then two merge levels
    down to (2nd+3rd largest) per block.
  - TensorE: block sums via 16 strided-rhs f32 matmuls against a 0/1 matrix
    (exact f32 mean path) + batch accumulation of the quantile stat in PSUM.
  - Output: ratio computed in a row-duplicated [128,512] layout, expanded and
    written as 32 full-partition 1 MiB DMAs.
"""

import os

import numpy as np

# ---- problem constants (hardcoded; kernel.py must be self-contained) ----
B = 16          # batch
H = 2048
W = 2048
BS = 4          # block size
NCORES = 8
HS = H // NCORES            # 256 rows per core
NBH = HS // BS              # 64 block rows per core
NBW = W // BS               # 512 block cols
ROWS = B * HS               # 4096 rows in a per-core slab
NGROUPS = 8                 # groups per core; each = 2 batches x 256 rows
GB = B // NGROUPS           # 2 batches per group
DECAY = 0.99
ALPHA = 0.1
EPS = 1e-5
C_MEAN = (1.0 - DECAY) / (BS * BS * B)    # fold mean-over-16-elems and batch
C_QUANT = (1.0 - DECAY) * 0.5 / B         # fold 0.5*(m2+m3) and batch mean

_CACHE = {}


def _split_multi_waits(nc):
    """This walrus build encodes at most ONE sync wait per instruction.
    Tile attaches several. Hoist excess waits onto same-engine NOPs placed
    immediately before the owning instruction (same engine stream => same
    semantics)."""
    import concourse.mybir as mybir

    plans = []  # (inst_name, extra_waits)
    for f in nc.m.functions:
        for bb in f.blocks:
            for inst in bb.instructions:
                si = getattr(inst, "sync_info", None)
                waits = list(si.on_wait) if (si and si.on_wait) else []
                if len(waits) > 1:
                    si.on_wait = [waits[-1]]
                    plans.append((inst.name, waits[:-1]))

    if not plans:
        return

    nop_for = {}
    stray = set()
    for iname, extra in plans:
        nops = []
        for w in extra:
            nop = nc.engines[nc.inst_map[iname].engine].nop(nofuse=True).ins
            nop.sync_info = mybir.SyncInfo(on_wait=[w], on_update=[])
            nops.append(nop)
            stray.add(nop.name)
        nop_for[iname] = nops

    for f in nc.m.functions:
        for bb in f.blocks:
            out = []
            changed = False
            for inst in bb.instructions:
                if inst.name in stray:
                    changed = True
                    continue
                if inst.name in nop_for:
                    out.extend(nop_for[inst.name])
                    changed = True
                out.append(inst)
            if changed:
                bb.instructions = out


def _build():
    """Builds the single-core Bass program (SPMD across 8 cores)."""
    from contextlib import ExitStack

    import concourse.bass as bass
    import concourse.mybir as mybir
    import concourse.tile as tile

    f32 = mybir.dt.float32
    f16 = mybir.dt.float16
    MAX = mybir.AluOpType.max
    MIN = mybir.AluOpType.min
    MULT = mybir.AluOpType.mult
    ADD = mybir.AluOpType.add

    nc = bass.Bass("TRN2", target_bir_lowering=False, debug=False)

    x = nc.dram_tensor("x", [ROWS, W], f32, kind="ExternalInput").ap()
    ee = nc.dram_tensor("ee", [NBH, NBW], f32, kind="ExternalInput").ap()
    eq = nc.dram_tensor("eq", [NBH, NBW], f32, kind="ExternalInput").ap()
    # ones2[p, m] = (p % 64 == m // 2): batch-pair fold + row duplication
    ones2 = nc.dram_tensor("ones2", [128, 128], f32, kind="ExternalInput").ap()
    y = nc.dram_tensor("y", [ROWS, W], f32, kind="ExternalOutput").ap()

    # input: row = ((g*2 + b2)*64 + i)*4 + r; per (g, r): [128=(b2,i), 2048]
    xr = x.rearrange("(g b2 i r) w -> g r (b2 i) w", g=NGROUPS, b2=GB, i=NBH, r=BS)
    # output: row = b*256 + 4i + 2h + r2; per (b, h): [64, 2, 2048] = 128
    # outer steps zipped against the SBUF tile's 128 partitions (p = 2i + r2)
    y5 = y.rearrange("(b i h r2) w -> b h i r2 w", b=B, i=NBH, h=2, r2=2)

    with tile.TileContext(nc) as tc, ExitStack() as ctx:
        pool = ctx.enter_context(tc.tile_pool(name="work", bufs=1))
        ppool = ctx.enter_context(tc.tile_pool(name="acc", bufs=1, space="PSUM"))

        psum_s = ppool.tile([128, NBW], f32, tag="ps")
        psum_q = ppool.tile([128, NBW], f32, tag="pq")

        ones_sb = pool.tile([128, 128], f32, tag="ones")
        nc.sync.dma_start(ones_sb[:, :], ones2)

        def tt(dst, a, bb, op):
            nc.vector.tensor_tensor(dst, a, bb, op)

        for g in range(NGROUPS):
            rts = []
            for r in range(BS):
                rt = pool.tile([128, W], f32, tag=f"r{r}", bufs=2)
                nc.sync.dma_start(rt[:, :], xr[g, r])
                rts.append(rt)

            # exact f32 block sums, split DVE/PE: rows 0+1 summed and
            # window-reduced on VectorE; rows 2+3 via strided-rhs matmuls.
            s01 = pool.tile([128, W], f32, tag="s01", bufs=1)
            tt(s01[:, :], rts[0][:, :], rts[1][:, :], ADD)
            ws1 = pool.tile([128, NBW], f32, tag="ws1", bufs=2)
            nc.vector.reduce_sum(
                ws1[:, :], s01.rearrange("p (j c) -> p j c", c=BS),
                axis=mybir.AxisListType.X,
            )
            k0 = g * 9
            nc.tensor.matmul(
                psum_s[:, :], lhsT=ones_sb[:, :], rhs=ws1[:, :],
                start=(k0 == 0), stop=False,
            )
            for r in (2, 3):
                rv = rts[r].rearrange("p (j c) -> p j c", c=BS)
                for c in range(BS):
                    k = k0 + 1 + (r - 2) * BS + c
                    nc.tensor.matmul(
                        psum_s[:, :], lhsT=ones_sb[:, :], rhs=rv[:, :, c],
                        start=False, stop=(k == NGROUPS * 9 - 1),
                    )

            # fp16 casts on ScalarE
            bts = []
            for r in range(BS):
                bt = pool.tile([128, W], f16, tag=f"b{r}", bufs=2)
                nc.scalar.copy(bt[:, :], rts[r][:, :])
                bts.append(bt)
            b0, b1, b2_, b3 = bts

            # vertical sorted-3 per column: m >= s2 >= t3 (fp16, 2x)
            v1 = pool.tile([128, W], f16, tag="big", bufs=7)
            tt(v1[:, :], b0[:, :], b1[:, :], MAX)
            w1v = pool.tile([128, W], f16, tag="big", bufs=7)
            tt(w1v[:, :], b0[:, :], b1[:, :], MIN)
            v2 = pool.tile([128, W], f16, tag="big", bufs=7)
            tt(v2[:, :], b2_[:, :], b3[:, :], MAX)
            w2v = pool.tile([128, W], f16, tag="big", bufs=7)
            tt(w2v[:, :], b2_[:, :], b3[:, :], MIN)
            m = pool.tile([128, W], f16, tag="big", bufs=7)
            tt(m[:, :], v1[:, :], v2[:, :], MAX)
            t1 = pool.tile([128, W], f16, tag="big", bufs=7)
            tt(t1[:, :], v1[:, :], v2[:, :], MIN)
            t2 = pool.tile([128, W], f16, tag="big", bufs=7)
            tt(t2[:, :], w1v[:, :], w2v[:, :], MAX)
            s2 = pool.tile([128, W], f16, tag="big", bufs=7)
            tt(s2[:, :], t1[:, :], t2[:, :], MAX)
            t3 = pool.tile([128, W], f16, tag="big", bufs=7)
            tt(t3[:, :], t1[:, :], t2[:, :], MIN)

            # ScalarE de-interleave of m/s2/t3 into even/odd planes
            HW2 = W // 2

            def deint(src, w_out, tag, bufs):
                v = src.rearrange("p (j two) -> p j two", two=2)
                te = pool.tile([128, w_out], f16, tag=tag, bufs=bufs, name=f"{tag}e")
                nc.scalar.copy(te[:, :], v[:, :, 0])
                to = pool.tile([128, w_out], f16, tag=tag, bufs=bufs, name=f"{tag}o")
                nc.scalar.copy(to[:, :], v[:, :, 1])
                return te, to

            me, mo = deint(m, HW2, "eoa", 7)
            s2e, s2o = deint(s2, HW2, "eoa", 7)
            t3e, t3o = deint(t3, HW2, "eoa", 7)

            # level A: merge adjacent-column sorted-3 pairs -> top-3 (fp16 2x)
            p1 = pool.tile([128, HW2], f16, tag="mid", bufs=7)
            tt(p1[:, :], me[:, :], mo[:, :], MAX)
            u1 = pool.tile([128, HW2], f16, tag="mid", bufs=7)
            tt(u1[:, :], me[:, :], mo[:, :], MIN)
            u2 = pool.tile([128, HW2], f16, tag="mid", bufs=7)
            tt(u2[:, :], s2e[:, :], s2o[:, :], MAX)
            p2 = pool.tile([128, HW2], f16, tag="mid", bufs=7)
            tt(p2[:, :], u1[:, :], u2[:, :], MAX)
            w1 = pool.tile([128, HW2], f16, tag="mid", bufs=7)
            tt(w1[:, :], t3e[:, :], t3o[:, :], MAX)
            w2 = pool.tile([128, HW2], f16, tag="mid", bufs=7)
            tt(w2[:, :], me[:, :], s2o[:, :], MIN)
            w3 = pool.tile([128, HW2], f16, tag="mid", bufs=7)
            tt(w3[:, :], s2e[:, :], mo[:, :], MIN)
            w4 = pool.tile([128, HW2], f16, tag="mid", bufs=7)
            tt(w4[:, :], w2[:, :], w3[:, :], MAX)
            p3 = pool.tile([128, HW2], f16, tag="mid", bufs=7)
            tt(p3[:, :], w1[:, :], w4[:, :], MAX)

            # ScalarE de-interleave of p1/p2/p3
            p1e, p1o = deint(p1, NBW, "eob", 7)
            p2e, p2o = deint(p2, NBW, "eob", 7)
            p3e, p3o = deint(p3, NBW, "eob", 7)

            # level B: 2nd + 3rd largest of all 16 (fp16 2x, final add f32)
            z1 = pool.tile([128, NBW], f16, tag="small", bufs=6)
            tt(z1[:, :], p1e[:, :], p1o[:, :], MIN)
            z2 = pool.tile([128, NBW], f16, tag="small", bufs=6)
            tt(z2[:, :], p2e[:, :], p2o[:, :], MAX)
            c2 = pool.tile([128, NBW], f16, tag="small", bufs=6)
            tt(c2[:, :], z1[:, :], z2[:, :], MAX)
            z3 = pool.tile([128, NBW], f16, tag="small", bufs=6)
            tt(z3[:, :], p3e[:, :], p3o[:, :], MAX)
            z4 = pool.tile([128, NBW], f16, tag="small", bufs=6)
            tt(z4[:, :], p1e[:, :], p2o[:, :], MIN)
            z5 = pool.tile([128, NBW], f16, tag="small", bufs=6)
            tt(z5[:, :], p2e[:, :], p1o[:, :], MIN)
            z6 = pool.tile([128, NBW], f16, tag="small", bufs=6)
            tt(z6[:, :], z4[:, :], z5[:, :], MAX)
            c3 = pool.tile([128, NBW], f16, tag="small", bufs=6)
            tt(c3[:, :], z3[:, :], z6[:, :], MAX)
            qs = pool.tile([128, NBW], f32, tag="qs", bufs=2)
            tt(qs[:, :], c2[:, :], c3[:, :], ADD)

            nc.tensor.matmul(
                psum_q[:, :], lhsT=ones_sb[:, :], rhs=qs[:, :],
                start=(g == 0), stop=(g == NGROUPS - 1),
            )

        # ---- tail: EMA update + ratio + broadcast (row-duplicated layout) ----
        ee_sb = pool.tile([128, NBW], f32, tag="ee")
        nc.sync.dma_start(ee_sb[:, :], ee.unsqueeze(1).broadcast_to((NBH, 2, NBW)))
        eq_sb = pool.tile([128, NBW], f32, tag="eq")
        nc.sync.dma_start(eq_sb[:, :], eq.unsqueeze(1).broadcast_to((NBH, 2, NBW)))

        ee2 = pool.tile([128, NBW], f32, tag="ee2")
        nc.vector.tensor_scalar(ee2[:, :], ee_sb[:, :], DECAY, EPS, MULT, ADD)
        eq2 = pool.tile([128, NBW], f32, tag="eq2")
        nc.vector.tensor_scalar(eq2[:, :], eq_sb[:, :], DECAY, 0.0, MULT, ADD)

        den = pool.tile([128, NBW], f32, tag="den")
        nc.vector.scalar_tensor_tensor(
            den[:, :], psum_s[:, :], C_MEAN, ee2[:, :], op0=MULT, op1=ADD
        )
        num = pool.tile([128, NBW], f32, tag="num")
        nc.vector.scalar_tensor_tensor(
            num[:, :], psum_q[:, :], C_QUANT, eq2[:, :], op0=MULT, op1=ADD
        )
        rec = pool.tile([128, NBW], f32, tag="rec")
        nc.vector.reciprocal(rec[:, :], den[:, :])
        u = pool.tile([128, NBW], f32, tag="u")
        nc.vector.tensor_tensor(u[:, :], num[:, :], rec[:, :], MULT)

        # expand x4 along columns: u4[p, j*4 + c] = u[p, j]
        u4 = pool.tile([128, W], f32, tag="u4")
        u4v = u4.rearrange("p (j c) -> p j c", c=BS)
        for c in range(BS):
            nc.vector.tensor_copy(u4v[:, :, c], u[:, :])

        # 32 full-partition writes: batch x row-pair-half
        for b in range(B):
            for h in range(2):
                nc.sync.dma_start(y5[b, h], u4[:, :])

    _split_multi_waits(nc)
    return nc


def _get_nc():
    if "nc" not in _CACHE:
        _CACHE["nc"] = _build()
    return _CACHE["nc"]


def kernel(current_errors, ema_errors, ema_quantile):
    from concourse.bass_utils import run_bass_kernel_spmd

    x = np.asarray(current_errors, dtype=np.float32).reshape(B, H, W)
    ee = np.asarray(ema_errors, dtype=np.float32).reshape(H // BS, W // BS)
    eq = np.asarray(ema_quantile, dtype=np.float32).reshape(H // BS, W // BS)

    # ones2[p, m] == 1 iff p % 64 == m // 2
    ones2 = np.zeros((128, 128), dtype=np.float32)
    p = np.arange(128)
    ones2[p, (p % NBH) * 2] = 1.0
    ones2[p, (p % NBH) * 2 + 1] = 1.0

    in_maps = []
    for k in range(NCORES):
        xs = np.ascontiguousarray(x[:, k * HS : (k + 1) * HS, :]).reshape(ROWS, W)
        ees = np.ascontiguousarray(ee[k * NBH : (k + 1) * NBH, :])
        eqs = np.ascontiguousarray(eq[k * NBH : (k + 1) * NBH, :])
        in_maps.append({"x": xs, "ee": ees, "eq": eqs, "ones2": ones2})

    nc = _get_nc()
    trace = bool(int(os.environ.get("KERNEL_TRACE", "0")))
    res = run_bass_kernel_spmd(
        nc, in_maps, core_ids=list(range(NCORES)), trace=trace
    )
    _CACHE["last_results"] = res

    out = np.empty((B, 1, H, W), dtype=np.float32)
    for k in range(NCORES):
        out[:, 0, k * HS : (k + 1) * HS, :] = res.results[k]["y"].reshape(B, HS, W)
    return out
